# revision 20
# baseline (speedup 1.0000x reference)
"""MoE (DeepSeek-style gate + 32 routed SwiGLU experts + shared expert) on 8 trn2 cores.

Strategy: data-parallel over tokens (batch dim 8 -> 1 slab per core), expert
weights replicated.  Two device launches per call:

  1. gate kernel: computes dense combine-weights cw[T, E] (softmax + grouped
     top-k routing) on device.  Plain fp32 matmuls (routing decisions are
     tie-sensitive; bf16 scores flip ~1% of token expert sets, fp32r flips
     ~10/32k - both blow the error budget).  Weights-stationary orientation:
     lhsT=gw [128,32], rhs=x [128,512], scores land transposed [E, tokens].
  2. main kernel: per expert, gathers its tokens (host builds the gather
     layout from cw - pure data movement), runs the SwiGLU expert MLP in
     bf16 (fp32 PSUM accumulate), scales by routing weight, scatters rows
     into a slot buffer, then combines slots + shared-expert output +
     (b2/sb2 via a small cw @ [b2;sb2] matmul) into y.

All tensors the device streams at full rate are PRE-TILED on the host into
their exact SBUF layout so every DMA line is >=2 KB contiguous per partition
(1 KB-line layouts measured ~130 GB/s vs ~358 peak and stalled the PE).
All arithmetic happens on device; the host only reshapes/permutes/casts data.
"""

import sys

sys.path.insert(0, "/opt/trn_rl_repo")

import numpy as np
import ml_dtypes

import concourse.bacc as bacc
import concourse.mybir as mybir
import concourse.tile as tile
from concourse import bass
from concourse.bass_utils import run_bass_kernel_spmd
from concourse.masks import make_identity

NCORES = 8
DIM = 1024
INTER = 512
E = 32
TOPK = 4
GROUPS = 8
TOPK_G = 4
SINTER = 1024
P = 128
KD = DIM // P     # 8 k-tiles over dim
KI = INTER // P   # 4 k-tiles over inter
KS = SINTER // P  # 8 k-tiles over shared inter

F32 = mybir.dt.float32
F32R = mybir.dt.float32r
BF16 = mybir.dt.bfloat16
F16 = mybir.dt.float16
I32 = mybir.dt.int32
AF = mybir.ActivationFunctionType
OP = mybir.AluOpType
AX = mybir.AxisListType
NPBF16 = ml_dtypes.bfloat16


def _chunks(total, size):
    out = []
    off = 0
    while off < total:
        out.append((off, min(size, total - off)))
        off += size
    return out


def _tile_k(w, p=P):
    """[K, N] -> [p, (K//p)*N]: row k*p+q lands at partition q, free k*N..k*N+N."""
    K, N = w.shape
    return np.ascontiguousarray(
        w.reshape(K // p, p, N).transpose(1, 0, 2).reshape(p, (K // p) * N))


def build_gate(T):
    nc = bacc.Bacc("TRN2", target_bir_lowering=False)
    NB = T // 512
    xTt = nc.dram_tensor("xTt", [NB, P, KD * 512], F32, kind="ExternalInput")
    gwd = nc.dram_tensor("gwd", [P, KD * E], F32, kind="ExternalInput")
    gb = nc.dram_tensor("gb", [E, 1], F32, kind="ExternalInput")
    cw = nc.dram_tensor("cw", [T, E], F32, kind="ExternalOutput")
    with tile.TileContext(nc) as tc:
        with tc.tile_pool(name="cst", bufs=1) as cst, \
             tc.tile_pool(name="xp", bufs=3) as xp, \
             tc.tile_pool(name="sb", bufs=2) as sb, \
             tc.tile_pool(name="ps", bufs=2, space="PSUM") as ps, \
             tc.tile_pool(name="pt", bufs=4, space="PSUM") as pt:
            gwt = cst.tile([P, KD, E], F32)
            nc.sync.dma_start(out=gwt[:], in_=gwd.ap().rearrange("p (k e) -> p k e", k=KD))
            gbt = cst.tile([E, 1], F32)
            nc.sync.dma_start(out=gbt[:], in_=gb.ap())
            ident = cst.tile([P, P], F32)
            make_identity(nc, ident[:])
            for b in range(NB):
                n0 = b * 512
                xt = xp.tile([P, KD, 512], F32, tag="xt")
                nc.sync.dma_start(out=xt[:], in_=xTt.ap()[b].rearrange("p (k n) -> p k n", k=KD))
                # scores transposed: [E, 512] = gw.T @ x
                s = ps.tile([E, 512], F32, tag="s")
                for k in range(KD):
                    nc.tensor.matmul(out=s[:], lhsT=gwt[:, k, :], rhs=xt[:, k, :],
                                     start=(k == 0), stop=(k == KD - 1))
                sc = sb.tile([E, 512], F32, tag="sc")
                nc.scalar.activation(sc[:], s[:], AF.Identity, bias=gbt[:, 0:1], scale=1.0)
                # PE-transpose back to [tokens, E], 4 chunks of 128
                s4 = sb.tile([P, 4, E], F32, tag="s4")
                for c in range(4):
                    pst = pt.tile([P, E], F32, tag="pst")
                    nc.tensor.transpose(out=pst[:], in_=sc[:, c * P:(c + 1) * P],
                                        identity=ident[:E, :E])
                    nc.scalar.copy(s4[:, c, :], pst[:])
                # batched softmax over the innermost 32
                negmax = sb.tile([P, 4], F32, tag="negmax")
                nc.vector.tensor_reduce(out=negmax[:], in_=s4[:], op=OP.max, axis=AX.X,
                                        negate=True)
                sh = sb.tile([P, 4, E], F32, tag="sh")
                nc.vector.tensor_tensor(out=sh[:], in0=s4[:],
                                        in1=negmax[:].unsqueeze(2).to_broadcast([P, 4, E]),
                                        op=OP.add)
                et = sb.tile([P, 4, E], F32, tag="et")
                nc.scalar.activation(et[:], sh[:], AF.Exp)
                ssum = sb.tile([P, 4], F32, tag="ssum")
                nc.vector.reduce_sum(out=ssum[:], in_=et[:], axis=AX.X)
                rsum = sb.tile([P, 4], F32, tag="rsum")
                nc.vector.reciprocal(rsum[:], ssum[:])
                pr = sb.tile([P, 4, E], F32, tag="pr")
                nc.vector.tensor_tensor(out=pr[:], in0=et[:],
                                        in1=rsum[:].unsqueeze(2).to_broadcast([P, 4, E]),
                                        op=OP.mult)
                # group scores: sum of top-2 within each group of 4
                # top2sum(a,b,c,d) = max(a+b, c+d, max(a,b)+max(c,d))
                g = pr[:].rearrange("p c (g f) -> p c g f", f=4)
                ga = sb.tile([P, 4, GROUPS], F32, tag="ga")
                gbv = sb.tile([P, 4, GROUPS], F32, tag="gbv")
                m1 = sb.tile([P, 4, GROUPS], F32, tag="m1")
                gsc = sb.tile([P, 4, GROUPS], F32, tag="gsc")
                nc.vector.tensor_add(ga[:], g[:, :, :, 0], g[:, :, :, 1])
                nc.vector.tensor_add(gbv[:], g[:, :, :, 2], g[:, :, :, 3])
                nc.vector.tensor_tensor(out=m1[:], in0=g[:, :, :, 0], in1=g[:, :, :, 1],
                                        op=OP.max)
                nc.vector.tensor_tensor(out=gsc[:], in0=g[:, :, :, 2], in1=g[:, :, :, 3],
                                        op=OP.max)
                nc.vector.tensor_add(m1[:], m1[:], gsc[:])
                nc.vector.tensor_tensor(out=ga[:], in0=ga[:], in1=gbv[:], op=OP.max)
                nc.vector.tensor_tensor(out=gsc[:], in0=ga[:], in1=m1[:], op=OP.max)
                # keep top-4 groups, then top-4 experts within kept groups
                keep = sb.tile([P, 4, GROUPS], F32, tag="keep")
                for c in range(4):
                    srt = sb.tile([P, 8], F32, tag="srt")
                    nc.vector.max(srt[:], gsc[:, c, :])
                    nc.vector.tensor_scalar(keep[:, c, :], gsc[:, c, :], srt[:, 3:4],
                                            None, op0=OP.is_ge)
                masked = sb.tile([P, 4, E], F32, tag="masked")
                nc.vector.tensor_tensor(
                    out=masked[:].rearrange("p c (g f) -> p c g f", f=4),
                    in0=g,
                    in1=keep[:].unsqueeze(3).to_broadcast([P, 4, GROUPS, 4]),
                    op=OP.mult,
                )
                cwt = sb.tile([P, 4, E], F32, tag="cwt")
                for c in range(4):
                    srt2 = sb.tile([P, 8], F32, tag="srt2")
                    nc.vector.max(srt2[:], masked[:, c, :])
                    nc.vector.tensor_scalar(cwt[:, c, :], masked[:, c, :], srt2[:, 3:4],
                                            None, op0=OP.is_ge)
                nc.vector.tensor_mul(cwt[:], cwt[:], masked[:])
                nc.sync.dma_start(
                    out=cw.ap()[n0:n0 + 512, :].rearrange("(c p) e -> p c e", p=P),
                    in_=cwt[:])
    return nc


def build_main(T, seg_len, nblk, nfull, tmax, skip_bias2=False):
    """seg_len[e]: padded token count for expert e (same across cores).
    zbuf rows: slot k of token t at k*T+t, dummy rows (padding pairs) at 4*T.
    nblk: total 512-token blocks (incl. tails), nfull: full blocks only,
    tmax: padded max tail length."""
    nc = bacc.Bacc("TRN2", target_bir_lowering=False)
    NB = T // 512
    xTt = nc.dram_tensor("xTt", [NB, P, KD * 512], BF16, kind="ExternalInput")
    xgt = nc.dram_tensor("xgt", [max(nfull, 1), P, KD * 512], BF16, kind="ExternalInput")
    xgl = nc.dram_tensor("xgl", [E, P, KD * max(tmax, 4)], BF16, kind="ExternalInput")
    meta = nc.dram_tensor("meta", [P, nblk * 8], I32, kind="ExternalInput")
    cwT1 = nc.dram_tensor("cwT1", [E + 1, T], F32R, kind="ExternalInput")
    b2a = nc.dram_tensor("b2a", [E + 1, DIM], F32R, kind="ExternalInput")
    w1 = nc.dram_tensor("w1", [E, P, KD * INTER], BF16, kind="ExternalInput")
    w3 = nc.dram_tensor("w3", [E, P, KD * INTER], BF16, kind="ExternalInput")
    w2 = nc.dram_tensor("w2", [E, P, KI * DIM], BF16, kind="ExternalInput")
    b1d = nc.dram_tensor("b1d", [P, E * KI], F32, kind="ExternalInput")
    b3d = nc.dram_tensor("b3d", [P, E * KI], F32, kind="ExternalInput")
    sw1 = nc.dram_tensor("sw1", [2, P, 4 * SINTER], BF16, kind="ExternalInput")
    sw3 = nc.dram_tensor("sw3", [2, P, 4 * SINTER], BF16, kind="ExternalInput")
    sw2 = nc.dram_tensor("sw2", [2, P, 4 * DIM], BF16, kind="ExternalInput")
    sb1 = nc.dram_tensor("sb1", [P, KS], F32, kind="ExternalInput")
    sb3 = nc.dram_tensor("sb3", [P, KS], F32, kind="ExternalInput")
    y = nc.dram_tensor("y", [T, DIM], F32, kind="ExternalOutput")
    zbuf = nc.dram_tensor("zbuf", [4 * T + P, DIM], F16)

    from contextlib import ExitStack
    with tile.TileContext(nc) as tc:
        with ExitStack() as ctx:
            cst = ctx.enter_context(tc.tile_pool(name="cst", bufs=1))
            wp = ctx.enter_context(tc.tile_pool(name="wp", bufs=3))
            sp = ctx.enter_context(tc.tile_pool(name="sp", bufs=1))
            xp = ctx.enter_context(tc.tile_pool(name="xp", bufs=3))
            hp = ctx.enter_context(tc.tile_pool(name="hp", bufs=2))
            ep = ctx.enter_context(tc.tile_pool(name="ep", bufs=2))
            zp = ctx.enter_context(tc.tile_pool(name="zp", bufs=2))
            cp = ctx.enter_context(tc.tile_pool(name="cp", bufs=3))
            pp1 = ctx.enter_context(tc.tile_pool(name="pp1", bufs=2, space="PSUM"))
            pp2 = ctx.enter_context(tc.tile_pool(name="pp2", bufs=3, space="PSUM"))

            metat = cst.tile([P, nblk, 8], I32)
            nc.sync.dma_start(out=metat[:], in_=meta.ap().rearrange("p (j m) -> p j m", m=8))
            b1all = cst.tile([P, E, KI], F32)
            nc.sync.dma_start(out=b1all[:], in_=b1d.ap().rearrange("p (e m) -> p e m", m=KI))
            b3all = cst.tile([P, E, KI], F32)
            nc.sync.dma_start(out=b3all[:], in_=b3d.ap().rearrange("p (e m) -> p e m", m=KI))

            def up_proj(xt, w1t, w3t, e, ht, m, nlen):
                """ht[:, m, :nlen] = silu(w1^T x + b1) * (w3^T x + b3) for inter tile m."""
                ps1 = pp1.tile([P, 512], F32, tag="ps1")
                for k in range(KD):
                    nc.tensor.matmul(out=ps1[:, :nlen], lhsT=w1t[:, k, m * P:(m + 1) * P],
                                     rhs=xt[:, k, :nlen], start=(k == 0), stop=(k == KD - 1))
                ps3 = pp1.tile([P, 512], F32, tag="ps3")
                for k in range(KD):
                    nc.tensor.matmul(out=ps3[:, :nlen], lhsT=w3t[:, k, m * P:(m + 1) * P],
                                     rhs=xt[:, k, :nlen], start=(k == 0), stop=(k == KD - 1))
                hs = ep.tile([P, 512], F32, tag="hs")
                nc.scalar.activation(hs[:, :nlen], ps1[:, :nlen], AF.Silu,
                                     bias=b1all[:, e, m:m + 1], scale=1.0)
                h3 = ep.tile([P, 512], F32, tag="h3")
                nc.scalar.activation(h3[:, :nlen], ps3[:, :nlen], AF.Identity,
                                     bias=b3all[:, e, m:m + 1], scale=1.0)
                nc.vector.tensor_mul(ht[:, m, :nlen], hs[:, :nlen], h3[:, :nlen])

            # ---------------- phase A: routed experts ----------------
            order = [e for e in range(E) if seg_len[e] > 0]
            shared_tiles = {}

            def load_shared():
                tiles = {}
                for name, src in (("s1", sw1), ("s3", sw3), ("s2", sw2)):
                    for half in range(2):
                        t = sp.tile([P, 4, SINTER], BF16, tag=f"{name}{half}")
                        nc.scalar.dma_start(out=t[:], in_=src.ap()[half].rearrange("p (k i) -> p k i", k=4))
                        tiles[f"{name}{half}"] = t
                shared_tiles.update(tiles)

            jblk = 0
            jfull = 0
            for ei, e in enumerate(order):
                e = int(e)
                if ei == len(order) - 2:
                    # prefetch shared-expert weights during the tail of phase A
                    load_shared()
                w1t = wp.tile([P, KD, INTER], BF16, tag="w1e")
                w3t = wp.tile([P, KD, INTER], BF16, tag="w3e")
                if ei == 0:
                    # fine-grained first loads: PE can start on k-slice 0 asap
                    for k in range(KD):
                        nc.scalar.dma_start(
                            out=w1t[:, k, :],
                            in_=w1.ap()[e].rearrange("p (k i) -> p k i", k=KD)[:, k, :])
                    for k in range(KD):
                        nc.scalar.dma_start(
                            out=w3t[:, k, :],
                            in_=w3.ap()[e].rearrange("p (k i) -> p k i", k=KD)[:, k, :])
                else:
                    nc.scalar.dma_start(out=w1t[:], in_=w1.ap()[e].rearrange("p (k i) -> p k i", k=KD))
                    nc.scalar.dma_start(out=w3t[:], in_=w3.ap()[e].rearrange("p (k i) -> p k i", k=KD))
                w2t = wp.tile([P, KI, DIM], BF16, tag="w2e")
                nc.scalar.dma_start(out=w2t[:], in_=w2.ap()[e].rearrange("p (k d) -> p k d", k=KI))
                for (n0, nlen) in _chunks(int(seg_len[e]), 512):
                    if nlen == 512:
                        xt = xp.tile([P, KD, 512], BF16, tag="xg")
                        if ei == 0 and n0 == 0:
                            for k in range(KD):
                                nc.sync.dma_start(
                                    out=xt[:, k, :],
                                    in_=xgt.ap()[jfull].rearrange("p (k n) -> p k n", k=KD)[:, k, :])
                        else:
                            nc.sync.dma_start(out=xt[:], in_=xgt.ap()[jfull].rearrange("p (k n) -> p k n", k=KD))
                        jfull += 1
                    else:
                        xt = xp.tile([P, KD, 512], BF16, tag="xg")
                        nc.sync.dma_start(
                            out=xt[:, :, :nlen],
                            in_=xgl.ap()[e].rearrange("p (k n) -> p k n", k=KD)[:, :, :nlen])
                    j = jblk
                    jblk += 1
                    nch = (nlen + P - 1) // P
                    ht = hp.tile([P, KS, 512], BF16, tag="ht")
                    for m in range(KI):
                        up_proj(xt, w1t, w3t, e, ht, m, nlen)
                    for c in range(nch):
                        cl = min(P, nlen - c * P)
                        zt = zp.tile([P, DIM], F16, tag="zt")
                        for h in range(2):
                            psz = pp2.tile([P, 512], F32, tag="psz")
                            for k in range(KI):
                                nc.tensor.matmul(out=psz[:cl, :],
                                                 lhsT=ht[:, k, c * P:c * P + cl],
                                                 rhs=w2t[:, k, h * 512:(h + 1) * 512],
                                                 start=(k == 0), stop=(k == KI - 1))
                            nc.scalar.activation(
                                zt[:cl, h * 512:(h + 1) * 512], psz[:cl, :],
                                AF.Copy, scale=metat[:cl, j, c:c + 1].bitcast(F32))
                        nc.gpsimd.indirect_dma_start(
                            out=zbuf.ap(),
                            out_offset=bass.IndirectOffsetOnAxis(
                                ap=metat[:cl, j, 4 + c:5 + c], axis=0),
                            in_=zt[:cl, :],
                            in_offset=None,
                        )

            # ------- phase B+C fused: shared expert + combine per 512 tokens -------
            if not shared_tiles:
                load_shared()
            s1a, s1b = shared_tiles["s10"], shared_tiles["s11"]
            s3a, s3b = shared_tiles["s30"], shared_tiles["s31"]
            s2a, s2b = shared_tiles["s20"], shared_tiles["s21"]
            sb1t = cst.tile([P, KS], F32)
            nc.sync.dma_start(out=sb1t[:], in_=sb1.ap())
            sb3t = cst.tile([P, KS], F32)
            nc.sync.dma_start(out=sb3t[:], in_=sb3.ap())
            if not skip_bias2:
                b2t = cst.tile([E + 1, DIM], F32R)
                nc.sync.dma_start(out=b2t[:], in_=b2a.ap())

            for b in range(NB):
                n0 = b * 512
                xt = xp.tile([P, KD, 512], BF16, tag="xg")
                nc.sync.dma_start(out=xt[:], in_=xTt.ap()[b].rearrange("p (k n) -> p k n", k=KD))
                ht = hp.tile([P, KS, 512], BF16, tag="ht")
                for m in range(KS):
                    ps1 = pp1.tile([P, 512], F32, tag="ps1")
                    for k in range(KD):
                        w = s1a if k < 4 else s1b
                        nc.tensor.matmul(out=ps1[:], lhsT=w[:, k % 4, m * P:(m + 1) * P],
                                         rhs=xt[:, k, :], start=(k == 0), stop=(k == KD - 1))
                    ps3 = pp1.tile([P, 512], F32, tag="ps3")
                    for k in range(KD):
                        w = s3a if k < 4 else s3b
                        nc.tensor.matmul(out=ps3[:], lhsT=w[:, k % 4, m * P:(m + 1) * P],
                                         rhs=xt[:, k, :], start=(k == 0), stop=(k == KD - 1))
                    hs = ep.tile([P, 512], F32, tag="hs")
                    nc.scalar.activation(hs[:], ps1[:], AF.Silu,
                                         bias=sb1t[:, m:m + 1], scale=1.0)
                    h3 = ep.tile([P, 512], F32, tag="h3")
                    nc.scalar.activation(h3[:], ps3[:], AF.Identity,
                                         bias=sb3t[:, m:m + 1], scale=1.0)
                    nc.vector.tensor_mul(ht[:, m, :], hs[:], h3[:])
                if not skip_bias2:
                    cwb = cp.tile([E + 1, 512], F32R, tag="cwb")
                    nc.sync.dma_start(out=cwb[:], in_=cwT1.ap()[:, n0:n0 + 512])
                for c in range(4):
                    t0 = n0 + c * P
                    yt = cp.tile([P, DIM], F32, tag="yt")
                    for h in range(2):
                        psz = pp2.tile([P, 512], F32, tag="psz")
                        for k in range(KS):
                            w = s2a if k < 4 else s2b
                            nc.tensor.matmul(out=psz[:, :],
                                             lhsT=ht[:, k, c * P:(c + 1) * P],
                                             rhs=w[:, k % 4, h * 512:(h + 1) * 512],
                                             start=(k == 0),
                                             stop=(skip_bias2 and k == KS - 1))
                        if not skip_bias2:
                            nc.tensor.matmul(out=psz[:, :], lhsT=cwb[:, c * P:(c + 1) * P],
                                             rhs=b2t[:, h * 512:(h + 1) * 512],
                                             start=False, stop=True)
                        nc.scalar.copy(yt[:, h * 512:(h + 1) * 512], psz[:, :])
                    for k in range(4):
                        zt = zp.tile([P, DIM], F16, tag="zc")
                        nc.gpsimd.dma_start(out=zt[:], in_=zbuf.ap()[k * T + t0:k * T + t0 + P, :])
                        nc.vector.tensor_add(yt[:], yt[:], zt[:])
                    nc.gpsimd.dma_start(out=y.ap()[t0:t0 + P, :], in_=yt[:])
    return nc


def _host_route(cw, T):
    """From dense combine weights cw[T, E] build (per-core) routing lists.
    Returns tokens[e] (np arrays), weights[e], slot_of_pair[e]."""
    nz = cw > 0.0
    counts = nz.sum(1)
    toks, wts, slots = [], [], []
    slot_ctr = np.zeros(T, np.int64)
    # tokens with more than TOPK positives (ties): keep top TOPK by value
    drop = {}
    for t in np.nonzero(counts > TOPK)[0]:
        vals = cw[t]
        order = np.argsort(-vals, kind="stable")
        drop[t] = set(order[TOPK:][vals[order[TOPK:]] > 0].tolist())
    for e in range(E):
        tk = np.nonzero(nz[:, e])[0]
        if drop:
            tk = np.array([t for t in tk if not (t in drop and e in drop[t])], dtype=np.int64)
        toks.append(tk)
        wts.append(cw[tk, e])
        sl = slot_ctr[tk].copy()
        slot_ctr[tk] += 1
        slots.append(sl)
    return toks, wts, slots, slot_ctr


def kernel(x, gw, gb, w1, b1, w3, b3, w2, b2, sw1, sb1, sw3, sb3, sw2, sb2):
    x = np.ascontiguousarray(np.asarray(x, np.float32))
    B, S, _ = x.shape
    T = (B * S) // NCORES
    NB = T // 512
    xs = x.reshape(NCORES, T, DIM)
    xT = np.ascontiguousarray(xs.transpose(0, 2, 1))  # [NCORES, DIM, T]
    xTb = xT.astype(NPBF16)
    gb2d = np.ascontiguousarray(np.asarray(gb, np.float32).reshape(E, 1))

    def tile_x(xTc, dt):
        # [DIM, T] -> [NB, P, KD*512]: block b, partition p, free (k, n)
        return np.ascontiguousarray(
            xTc.reshape(KD, P, NB, 512).transpose(2, 1, 0, 3).reshape(NB, P, KD * 512)
        ).astype(dt)

    # ---- launch 1: gate (fp32) ----
    nc1 = build_gate(T)
    nc1.compile()
    gwd = _tile_k(np.asarray(gw, np.float32))  # [P, KD*E]
    in_maps = [{"xTt": tile_x(xT[c], np.float32), "gwd": gwd, "gb": gb2d}
               for c in range(NCORES)]
    res1 = run_bass_kernel_spmd(nc1, in_maps, core_ids=list(range(NCORES)))
    cw_all = np.concatenate([res1.results[c]["cw"] for c in range(NCORES)], 0)  # [B*S, E]

    # ---- host: rebalance token->core assignment (pure data movement) so
    # per-(core, expert) token counts are near-even; shrinks the shared
    # max-over-cores segment plan the device pads to.
    Tall = cw_all.shape[0]
    topi = np.argsort(-cw_all, kind="stable", axis=1)[:, :TOPK]  # >0 entries lead
    cnt2 = np.zeros((NCORES, E), np.int64)
    cap = np.full(NCORES, T, np.int64)
    totals = np.bincount(topi.ravel(), minlength=E)
    target = (totals + NCORES - 1) // NCORES
    perm = [[] for _ in range(NCORES)]
    rng_order = np.random.RandomState(0).permutation(Tall)
    for t in rng_order:
        es = topi[t]
        score = cnt2[:, es].sum(1) * 8 + (T - cap)
        score[cap == 0] = 1 << 60
        c = int(np.argmin(score))
        perm[c].append(t)
        cnt2[c, es] += 1
        cap[c] -= 1
    perm = [np.array(p, np.int64) for p in perm]
    xflat = x.reshape(B * S, DIM)
    xT = np.stack([np.ascontiguousarray(xflat[perm[c]].T) for c in range(NCORES)])
    xTb = xT.astype(NPBF16)
    cws = [np.ascontiguousarray(cw_all[perm[c]]) for c in range(NCORES)]

    # ---- host: build routing metadata (data movement only) ----
    routed = [_host_route(cws[c], T) for c in range(NCORES)]
    cnt = np.array([[len(routed[c][0][e]) for e in range(E)] for c in range(NCORES)])
    seg_len = cnt.max(0)  # shared static plan across cores
    seg_len = ((seg_len + 3) // 4) * 4  # even moving dim for the matmuls
    seg_start = np.concatenate([[0], np.cumsum(seg_len)]).astype(int)
    Lsum = int(seg_len.sum())
    DUMMY = 4 * T
    blocks = [(e, n0, nlen) for e in range(E) if seg_len[e] > 0
              for (n0, nlen) in _chunks(int(seg_len[e]), 512)]
    nblk = len(blocks)
    nfull = sum(1 for (_, _, nlen) in blocks if nlen == 512)
    tmax = max([nlen for (_, _, nlen) in blocks if nlen < 512], default=4)

    xgts, xgls, metas, cwT1s = [], [], [], []
    for c in range(NCORES):
        toks, wts, slots, slot_ctr = routed[c]
        xg = np.zeros((DIM, Lsum), NPBF16)
        pwv = np.zeros((Lsum,), np.float32)
        sov = np.full((Lsum,), DUMMY, np.int32)
        pad_list = []
        for e in range(E):
            s0 = seg_start[e]
            n = len(toks[e])
            if n:
                xg[:, s0:s0 + n] = xTb[c][:, toks[e]]
                pwv[s0:s0 + n] = wts[e]
                sov[s0:s0 + n] = (slots[e] * T + toks[e]).astype(np.int32)
            pad_list.extend(range(s0 + n, s0 + int(seg_len[e])))
        # route missing (token, slot) pairs (from dropped ties) to padding pairs,
        # which compute exact zeros -> correct "no contribution" rows.
        miss = [(t, s) for t in np.nonzero(slot_ctr < TOPK)[0]
                for s in range(int(slot_ctr[t]), TOPK)]
        assert len(miss) <= len(pad_list), "not enough padding slots"
        for (t, s), j in zip(miss, pad_list):
            sov[j] = np.int32(s * T + t)
        # pre-tiled xg: full blocks [nfull, P, KD*512], tails [E, P, KD*tmax]
        xgt = np.zeros((max(nfull, 1), P, KD * 512), NPBF16)
        xgl = np.zeros((E, P, KD * max(tmax, 4)), NPBF16)
        jf = 0
        for (e, n0, nlen) in blocks:
            g0 = seg_start[e] + n0
            blk = xg[:, g0:g0 + nlen].reshape(KD, P, nlen).transpose(1, 0, 2)  # [P, KD, nlen]
            if nlen == 512:
                xgt[jf] = blk.reshape(P, KD * 512)
                jf += 1
            else:
                xgl[e].reshape(P, KD, max(tmax, 4))[:, :, :nlen] = blk
        # merged per-block metadata [P, nblk, 8]: [:,:,0:4]=pw bits, [:,:,4:8]=soff
        mt = np.zeros((P, nblk, 8), np.int32)
        mt[:, :, 4:8] = DUMMY
        for j, (e, n0, nlen) in enumerate(blocks):
            g0 = seg_start[e] + n0
            idx = np.arange(nlen)
            mt[idx % P, j, idx // P] = pwv[g0:g0 + nlen].view(np.int32)
            mt[idx % P, j, 4 + idx // P] = sov[g0:g0 + nlen]
        xgts.append(xgt)
        xgls.append(xgl)
        metas.append(np.ascontiguousarray(mt.reshape(P, nblk * 8)))
        cwT1s.append(np.ascontiguousarray(
            np.concatenate([cws[c].T, np.ones((1, T), np.float32)], 0)))

    b2a = np.ascontiguousarray(np.concatenate(
        [np.asarray(b2, np.float32), np.asarray(sb2, np.float32).reshape(1, DIM)], 0))

    # ---- launch 2: main (bf16 matmuls, fp32 accumulate) ----
    skip_bias2 = not b2a.any()
    nc2 = build_main(T, seg_len, nblk, nfull, tmax, skip_bias2=skip_bias2)
    nc2.compile()
    asf32 = lambda a: np.asarray(a, np.float32)
    w1d = np.stack([_tile_k(asf32(w1)[e]) for e in range(E)]).astype(NPBF16)
    w3d = np.stack([_tile_k(asf32(w3)[e]) for e in range(E)]).astype(NPBF16)
    w2d = np.stack([_tile_k(asf32(w2)[e]) for e in range(E)]).astype(NPBF16)
    sw1d = np.stack([_tile_k(asf32(sw1)[h * 512:(h + 1) * 512]) for h in range(2)]).astype(NPBF16)
    sw3d = np.stack([_tile_k(asf32(sw3)[h * 512:(h + 1) * 512]) for h in range(2)]).astype(NPBF16)
    sw2d = np.stack([_tile_k(asf32(sw2)[h * 512:(h + 1) * 512]) for h in range(2)]).astype(NPBF16)
    b1dd = np.ascontiguousarray(asf32(b1).reshape(E, KI, P).transpose(2, 0, 1).reshape(P, E * KI))
    b3dd = np.ascontiguousarray(asf32(b3).reshape(E, KI, P).transpose(2, 0, 1).reshape(P, E * KI))
    sb1d = np.ascontiguousarray(asf32(sb1).reshape(KS, P).T)
    sb3d = np.ascontiguousarray(asf32(sb3).reshape(KS, P).T)
    in_maps = [{
        "xTt": tile_x(xTb[c], NPBF16), "xgt": xgts[c], "xgl": xgls[c],
        "meta": metas[c], "cwT1": cwT1s[c], "b2a": b2a,
        "w1": w1d, "w3": w3d, "w2": w2d, "b1d": b1dd, "b3d": b3dd,
        "sw1": sw1d, "sw3": sw3d, "sw2": sw2d, "sb1": sb1d, "sb3": sb3d,
    } for c in range(NCORES)]
    res2 = run_bass_kernel_spmd(nc2, in_maps, core_ids=list(range(NCORES)))
    yfull = np.empty((B * S, DIM), np.float32)
    for c in range(NCORES):
        yfull[perm[c]] = res2.results[c]["y"]
    return yfull.reshape(B, S, DIM)


# revision 21
# speedup vs baseline: 1.0265x; 1.0265x over previous
"""MoE (DeepSeek-style gate + 32 routed SwiGLU experts + shared expert) on 8 trn2 cores.

Strategy: data-parallel over tokens (batch dim 8 -> 1 slab per core), expert
weights replicated.  Two device launches per call:

  1. gate kernel: computes dense combine-weights cw[T, E] (softmax + grouped
     top-k routing) on device.  Plain fp32 matmuls (routing decisions are
     tie-sensitive; bf16 scores flip ~1% of token expert sets, fp32r flips
     ~10/32k - both blow the error budget).  Weights-stationary orientation:
     lhsT=gw [128,32], rhs=x [128,512], scores land transposed [E, tokens].
  2. main kernel: per expert, gathers its tokens (host builds the gather
     layout from cw - pure data movement), runs the SwiGLU expert MLP in
     bf16 (fp32 PSUM accumulate), scales by routing weight, scatters rows
     into a slot buffer, then combines slots + shared-expert output +
     (b2/sb2 via a small cw @ [b2;sb2] matmul) into y.

All tensors the device streams at full rate are PRE-TILED on the host into
their exact SBUF layout so every DMA line is >=2 KB contiguous per partition
(1 KB-line layouts measured ~130 GB/s vs ~358 peak and stalled the PE).
All arithmetic happens on device; the host only reshapes/permutes/casts data.
"""

import sys

sys.path.insert(0, "/opt/trn_rl_repo")

import numpy as np
import ml_dtypes

import concourse.bacc as bacc
import concourse.mybir as mybir
import concourse.tile as tile
from concourse import bass
from concourse.bass_utils import run_bass_kernel_spmd
from concourse.masks import make_identity

NCORES = 8
DIM = 1024
INTER = 512
E = 32
TOPK = 4
GROUPS = 8
TOPK_G = 4
SINTER = 1024
P = 128
KD = DIM // P     # 8 k-tiles over dim
KI = INTER // P   # 4 k-tiles over inter
KS = SINTER // P  # 8 k-tiles over shared inter

F32 = mybir.dt.float32
F32R = mybir.dt.float32r
BF16 = mybir.dt.bfloat16
F16 = mybir.dt.float16
I32 = mybir.dt.int32
AF = mybir.ActivationFunctionType
OP = mybir.AluOpType
AX = mybir.AxisListType
NPBF16 = ml_dtypes.bfloat16


def _chunks(total, size):
    out = []
    off = 0
    while off < total:
        out.append((off, min(size, total - off)))
        off += size
    return out


def _tile_k(w, p=P):
    """[K, N] -> [p, (K//p)*N]: row k*p+q lands at partition q, free k*N..k*N+N."""
    K, N = w.shape
    return np.ascontiguousarray(
        w.reshape(K // p, p, N).transpose(1, 0, 2).reshape(p, (K // p) * N))


def build_gate(T):
    nc = bacc.Bacc("TRN2", target_bir_lowering=False)
    NB = T // 512
    xTt = nc.dram_tensor("xTt", [NB, P, KD * 512], F32, kind="ExternalInput")
    gwd = nc.dram_tensor("gwd", [P, KD * E], F32, kind="ExternalInput")
    gb = nc.dram_tensor("gb", [E, 1], F32, kind="ExternalInput")
    cw = nc.dram_tensor("cw", [T, E], F32, kind="ExternalOutput")
    with tile.TileContext(nc) as tc:
        with tc.tile_pool(name="cst", bufs=1) as cst, \
             tc.tile_pool(name="xp", bufs=3) as xp, \
             tc.tile_pool(name="sb", bufs=2) as sb, \
             tc.tile_pool(name="ps", bufs=2, space="PSUM") as ps, \
             tc.tile_pool(name="pt", bufs=4, space="PSUM") as pt:
            gwt = cst.tile([P, KD, E], F32)
            nc.sync.dma_start(out=gwt[:], in_=gwd.ap().rearrange("p (k e) -> p k e", k=KD))
            gbt = cst.tile([E, 1], F32)
            nc.sync.dma_start(out=gbt[:], in_=gb.ap())
            ident = cst.tile([P, P], F32)
            make_identity(nc, ident[:])
            for b in range(NB):
                n0 = b * 512
                xt = xp.tile([P, KD, 512], F32, tag="xt")
                nc.sync.dma_start(out=xt[:], in_=xTt.ap()[b].rearrange("p (k n) -> p k n", k=KD))
                # scores transposed: [E, 512] = gw.T @ x
                s = ps.tile([E, 512], F32, tag="s")
                for k in range(KD):
                    nc.tensor.matmul(out=s[:], lhsT=gwt[:, k, :], rhs=xt[:, k, :],
                                     start=(k == 0), stop=(k == KD - 1))
                sc = sb.tile([E, 512], F32, tag="sc")
                nc.scalar.activation(sc[:], s[:], AF.Identity, bias=gbt[:, 0:1], scale=1.0)
                # PE-transpose back to [tokens, E], 4 chunks of 128
                s4 = sb.tile([P, 4, E], F32, tag="s4")
                for c in range(4):
                    pst = pt.tile([P, E], F32, tag="pst")
                    nc.tensor.transpose(out=pst[:], in_=sc[:, c * P:(c + 1) * P],
                                        identity=ident[:E, :E])
                    nc.scalar.copy(s4[:, c, :], pst[:])
                # batched softmax over the innermost 32
                negmax = sb.tile([P, 4], F32, tag="negmax")
                nc.vector.tensor_reduce(out=negmax[:], in_=s4[:], op=OP.max, axis=AX.X,
                                        negate=True)
                sh = sb.tile([P, 4, E], F32, tag="sh")
                nc.vector.tensor_tensor(out=sh[:], in0=s4[:],
                                        in1=negmax[:].unsqueeze(2).to_broadcast([P, 4, E]),
                                        op=OP.add)
                et = sb.tile([P, 4, E], F32, tag="et")
                nc.scalar.activation(et[:], sh[:], AF.Exp)
                ssum = sb.tile([P, 4], F32, tag="ssum")
                nc.vector.reduce_sum(out=ssum[:], in_=et[:], axis=AX.X)
                rsum = sb.tile([P, 4], F32, tag="rsum")
                nc.vector.reciprocal(rsum[:], ssum[:])
                pr = sb.tile([P, 4, E], F32, tag="pr")
                nc.vector.tensor_tensor(out=pr[:], in0=et[:],
                                        in1=rsum[:].unsqueeze(2).to_broadcast([P, 4, E]),
                                        op=OP.mult)
                # group scores: sum of top-2 within each group of 4
                # top2sum(a,b,c,d) = max(a+b, c+d, max(a,b)+max(c,d))
                g = pr[:].rearrange("p c (g f) -> p c g f", f=4)
                ga = sb.tile([P, 4, GROUPS], F32, tag="ga")
                gbv = sb.tile([P, 4, GROUPS], F32, tag="gbv")
                m1 = sb.tile([P, 4, GROUPS], F32, tag="m1")
                gsc = sb.tile([P, 4, GROUPS], F32, tag="gsc")
                nc.vector.tensor_add(ga[:], g[:, :, :, 0], g[:, :, :, 1])
                nc.vector.tensor_add(gbv[:], g[:, :, :, 2], g[:, :, :, 3])
                nc.vector.tensor_tensor(out=m1[:], in0=g[:, :, :, 0], in1=g[:, :, :, 1],
                                        op=OP.max)
                nc.vector.tensor_tensor(out=gsc[:], in0=g[:, :, :, 2], in1=g[:, :, :, 3],
                                        op=OP.max)
                nc.vector.tensor_add(m1[:], m1[:], gsc[:])
                nc.vector.tensor_tensor(out=ga[:], in0=ga[:], in1=gbv[:], op=OP.max)
                nc.vector.tensor_tensor(out=gsc[:], in0=ga[:], in1=m1[:], op=OP.max)
                # keep top-4 groups, then top-4 experts within kept groups
                keep = sb.tile([P, 4, GROUPS], F32, tag="keep")
                for c in range(4):
                    srt = sb.tile([P, 8], F32, tag="srt")
                    nc.vector.max(srt[:], gsc[:, c, :])
                    nc.vector.tensor_scalar(keep[:, c, :], gsc[:, c, :], srt[:, 3:4],
                                            None, op0=OP.is_ge)
                masked = sb.tile([P, 4, E], F32, tag="masked")
                nc.vector.tensor_tensor(
                    out=masked[:].rearrange("p c (g f) -> p c g f", f=4),
                    in0=g,
                    in1=keep[:].unsqueeze(3).to_broadcast([P, 4, GROUPS, 4]),
                    op=OP.mult,
                )
                cwt = sb.tile([P, 4, E], F32, tag="cwt")
                for c in range(4):
                    srt2 = sb.tile([P, 8], F32, tag="srt2")
                    nc.vector.max(srt2[:], masked[:, c, :])
                    nc.vector.tensor_scalar(cwt[:, c, :], masked[:, c, :], srt2[:, 3:4],
                                            None, op0=OP.is_ge)
                nc.vector.tensor_mul(cwt[:], cwt[:], masked[:])
                nc.sync.dma_start(
                    out=cw.ap()[n0:n0 + 512, :].rearrange("(c p) e -> p c e", p=P),
                    in_=cwt[:])
    return nc


def build_main(T, seg_len, nblk, nfull, tmax, skip_bias2=False):
    """seg_len[e]: padded token count for expert e (same across cores).
    zbuf rows: slot k of token t at k*T+t, dummy rows (padding pairs) at 4*T.
    nblk: total 512-token blocks (incl. tails), nfull: full blocks only,
    tmax: padded max tail length."""
    nc = bacc.Bacc("TRN2", target_bir_lowering=False)
    NB = T // 512
    xTt = nc.dram_tensor("xTt", [NB, P, KD * 512], BF16, kind="ExternalInput")
    xgt = nc.dram_tensor("xgt", [max(nfull, 1), P, KD * 512], BF16, kind="ExternalInput")
    xgl = nc.dram_tensor("xgl", [E, P, KD * max(tmax, 4)], BF16, kind="ExternalInput")
    meta = nc.dram_tensor("meta", [P, nblk * 8], I32, kind="ExternalInput")
    cwT1 = nc.dram_tensor("cwT1", [E + 1, T], F32R, kind="ExternalInput")
    b2a = nc.dram_tensor("b2a", [E + 1, DIM], F32R, kind="ExternalInput")
    w1 = nc.dram_tensor("w1", [E, P, KD * INTER], BF16, kind="ExternalInput")
    w3 = nc.dram_tensor("w3", [E, P, KD * INTER], BF16, kind="ExternalInput")
    w2 = nc.dram_tensor("w2", [E, P, KI * DIM], BF16, kind="ExternalInput")
    b1d = nc.dram_tensor("b1d", [P, E * KI], F32, kind="ExternalInput")
    b3d = nc.dram_tensor("b3d", [P, E * KI], F32, kind="ExternalInput")
    sw1 = nc.dram_tensor("sw1", [2, P, 4 * SINTER], BF16, kind="ExternalInput")
    sw3 = nc.dram_tensor("sw3", [2, P, 4 * SINTER], BF16, kind="ExternalInput")
    sw2 = nc.dram_tensor("sw2", [2, P, 4 * DIM], BF16, kind="ExternalInput")
    sb1 = nc.dram_tensor("sb1", [P, KS], F32, kind="ExternalInput")
    sb3 = nc.dram_tensor("sb3", [P, KS], F32, kind="ExternalInput")
    y = nc.dram_tensor("y", [T, DIM], F32, kind="ExternalOutput")
    zbuf = nc.dram_tensor("zbuf", [4 * T + P, DIM], F16)

    from contextlib import ExitStack
    with tile.TileContext(nc) as tc:
        with ExitStack() as ctx:
            cst = ctx.enter_context(tc.tile_pool(name="cst", bufs=1))
            wp = ctx.enter_context(tc.tile_pool(name="wp", bufs=3))
            sp = ctx.enter_context(tc.tile_pool(name="sp", bufs=1))
            xp = ctx.enter_context(tc.tile_pool(name="xp", bufs=3))
            hp = ctx.enter_context(tc.tile_pool(name="hp", bufs=2))
            ep = ctx.enter_context(tc.tile_pool(name="ep", bufs=2))
            zp = ctx.enter_context(tc.tile_pool(name="zp", bufs=2))
            cp = ctx.enter_context(tc.tile_pool(name="cp", bufs=3))
            pp1 = ctx.enter_context(tc.tile_pool(name="pp1", bufs=2, space="PSUM"))
            pp2 = ctx.enter_context(tc.tile_pool(name="pp2", bufs=3, space="PSUM"))

            metat = cst.tile([P, nblk, 8], I32)
            nc.sync.dma_start(out=metat[:], in_=meta.ap().rearrange("p (j m) -> p j m", m=8))
            b1all = cst.tile([P, E, KI], F32)
            nc.sync.dma_start(out=b1all[:], in_=b1d.ap().rearrange("p (e m) -> p e m", m=KI))
            b3all = cst.tile([P, E, KI], F32)
            nc.sync.dma_start(out=b3all[:], in_=b3d.ap().rearrange("p (e m) -> p e m", m=KI))

            def up_proj(xt, w1t, w3t, e, ht, m, nlen):
                """ht[:, m, :nlen] = silu(w1^T x + b1) * (w3^T x + b3) for inter tile m."""
                ps1 = pp1.tile([P, 512], F32, tag="ps1")
                for k in range(KD):
                    nc.tensor.matmul(out=ps1[:, :nlen], lhsT=w1t[:, k, m * P:(m + 1) * P],
                                     rhs=xt[:, k, :nlen], start=(k == 0), stop=(k == KD - 1))
                ps3 = pp1.tile([P, 512], F32, tag="ps3")
                for k in range(KD):
                    nc.tensor.matmul(out=ps3[:, :nlen], lhsT=w3t[:, k, m * P:(m + 1) * P],
                                     rhs=xt[:, k, :nlen], start=(k == 0), stop=(k == KD - 1))
                hs = ep.tile([P, 512], F32, tag="hs")
                nc.scalar.activation(hs[:, :nlen], ps1[:, :nlen], AF.Silu,
                                     bias=b1all[:, e, m:m + 1], scale=1.0)
                h3 = ep.tile([P, 512], F32, tag="h3")
                nc.scalar.activation(h3[:, :nlen], ps3[:, :nlen], AF.Identity,
                                     bias=b3all[:, e, m:m + 1], scale=1.0)
                nc.vector.tensor_mul(ht[:, m, :nlen], hs[:, :nlen], h3[:, :nlen])

            # ---------------- phase A: routed experts ----------------
            order = [e for e in range(E) if seg_len[e] > 0]
            shared_tiles = {}

            def load_shared():
                tiles = {}
                for name, src in (("s1", sw1), ("s3", sw3), ("s2", sw2)):
                    for half in range(2):
                        t = sp.tile([P, 4, SINTER], BF16, tag=f"{name}{half}")
                        ap = src.ap()[half].rearrange("p (k i) -> p k i", k=4)
                        nc.sync.dma_start(out=t[:, 0:2, :], in_=ap[:, 0:2, :])
                        nc.scalar.dma_start(out=t[:, 2:4, :], in_=ap[:, 2:4, :])
                        tiles[f"{name}{half}"] = t
                shared_tiles.update(tiles)

            jblk = 0
            jfull = 0
            for ei, e in enumerate(order):
                e = int(e)
                if ei == len(order) - 2:
                    # prefetch shared-expert weights during the tail of phase A
                    load_shared()
                w1t = wp.tile([P, KD, INTER], BF16, tag="w1e")
                w3t = wp.tile([P, KD, INTER], BF16, tag="w3e")
                w1ap = w1.ap()[e].rearrange("p (k i) -> p k i", k=KD)
                w3ap = w3.ap()[e].rearrange("p (k i) -> p k i", k=KD)
                if ei == 0:
                    # fine-grained first loads: PE can start on k-slice 0 asap
                    for k in range(KD):
                        (nc.sync if k % 2 == 0 else nc.scalar).dma_start(
                            out=w1t[:, k, :], in_=w1ap[:, k, :])
                    for k in range(KD):
                        (nc.sync if k % 2 == 0 else nc.scalar).dma_start(
                            out=w3t[:, k, :], in_=w3ap[:, k, :])
                else:
                    nc.sync.dma_start(out=w1t[:, 0:4, :], in_=w1ap[:, 0:4, :])
                    nc.scalar.dma_start(out=w1t[:, 4:8, :], in_=w1ap[:, 4:8, :])
                    nc.sync.dma_start(out=w3t[:, 0:4, :], in_=w3ap[:, 0:4, :])
                    nc.scalar.dma_start(out=w3t[:, 4:8, :], in_=w3ap[:, 4:8, :])
                w2t = wp.tile([P, KI, DIM], BF16, tag="w2e")
                w2ap = w2.ap()[e].rearrange("p (k d) -> p k d", k=KI)
                nc.sync.dma_start(out=w2t[:, 0:2, :], in_=w2ap[:, 0:2, :])
                nc.scalar.dma_start(out=w2t[:, 2:4, :], in_=w2ap[:, 2:4, :])
                for (n0, nlen) in _chunks(int(seg_len[e]), 512):
                    if nlen == 512:
                        xt = xp.tile([P, KD, 512], BF16, tag="xg")
                        xap = xgt.ap()[jfull].rearrange("p (k n) -> p k n", k=KD)
                        if ei == 0 and n0 == 0:
                            for k in range(KD):
                                (nc.sync if k % 2 == 0 else nc.scalar).dma_start(
                                    out=xt[:, k, :], in_=xap[:, k, :])
                        else:
                            nc.sync.dma_start(out=xt[:, 0:4, :], in_=xap[:, 0:4, :])
                            nc.scalar.dma_start(out=xt[:, 4:8, :], in_=xap[:, 4:8, :])
                        jfull += 1
                    else:
                        xt = xp.tile([P, KD, 512], BF16, tag="xg")
                        nc.sync.dma_start(
                            out=xt[:, :, :nlen],
                            in_=xgl.ap()[e].rearrange("p (k n) -> p k n", k=KD)[:, :, :nlen])
                    j = jblk
                    jblk += 1
                    nch = (nlen + P - 1) // P
                    ht = hp.tile([P, KS, 512], BF16, tag="ht")
                    for m in range(KI):
                        up_proj(xt, w1t, w3t, e, ht, m, nlen)
                    for c in range(nch):
                        cl = min(P, nlen - c * P)
                        zt = zp.tile([P, DIM], F16, tag="zt")
                        for h in range(2):
                            psz = pp2.tile([P, 512], F32, tag="psz")
                            for k in range(KI):
                                nc.tensor.matmul(out=psz[:cl, :],
                                                 lhsT=ht[:, k, c * P:c * P + cl],
                                                 rhs=w2t[:, k, h * 512:(h + 1) * 512],
                                                 start=(k == 0), stop=(k == KI - 1))
                            nc.scalar.activation(
                                zt[:cl, h * 512:(h + 1) * 512], psz[:cl, :],
                                AF.Copy, scale=metat[:cl, j, c:c + 1].bitcast(F32))
                        nc.gpsimd.indirect_dma_start(
                            out=zbuf.ap(),
                            out_offset=bass.IndirectOffsetOnAxis(
                                ap=metat[:cl, j, 4 + c:5 + c], axis=0),
                            in_=zt[:cl, :],
                            in_offset=None,
                        )

            # ------- phase B+C fused: shared expert + combine per 512 tokens -------
            if not shared_tiles:
                load_shared()
            s1a, s1b = shared_tiles["s10"], shared_tiles["s11"]
            s3a, s3b = shared_tiles["s30"], shared_tiles["s31"]
            s2a, s2b = shared_tiles["s20"], shared_tiles["s21"]
            sb1t = cst.tile([P, KS], F32)
            nc.sync.dma_start(out=sb1t[:], in_=sb1.ap())
            sb3t = cst.tile([P, KS], F32)
            nc.sync.dma_start(out=sb3t[:], in_=sb3.ap())
            if not skip_bias2:
                b2t = cst.tile([E + 1, DIM], F32R)
                nc.sync.dma_start(out=b2t[:], in_=b2a.ap())

            for b in range(NB):
                n0 = b * 512
                xt = xp.tile([P, KD, 512], BF16, tag="xg")
                xap = xTt.ap()[b].rearrange("p (k n) -> p k n", k=KD)
                nc.sync.dma_start(out=xt[:, 0:4, :], in_=xap[:, 0:4, :])
                nc.scalar.dma_start(out=xt[:, 4:8, :], in_=xap[:, 4:8, :])
                ht = hp.tile([P, KS, 512], BF16, tag="ht")
                for m in range(KS):
                    ps1 = pp1.tile([P, 512], F32, tag="ps1")
                    for k in range(KD):
                        w = s1a if k < 4 else s1b
                        nc.tensor.matmul(out=ps1[:], lhsT=w[:, k % 4, m * P:(m + 1) * P],
                                         rhs=xt[:, k, :], start=(k == 0), stop=(k == KD - 1))
                    ps3 = pp1.tile([P, 512], F32, tag="ps3")
                    for k in range(KD):
                        w = s3a if k < 4 else s3b
                        nc.tensor.matmul(out=ps3[:], lhsT=w[:, k % 4, m * P:(m + 1) * P],
                                         rhs=xt[:, k, :], start=(k == 0), stop=(k == KD - 1))
                    hs = ep.tile([P, 512], F32, tag="hs")
                    nc.scalar.activation(hs[:], ps1[:], AF.Silu,
                                         bias=sb1t[:, m:m + 1], scale=1.0)
                    h3 = ep.tile([P, 512], F32, tag="h3")
                    nc.scalar.activation(h3[:], ps3[:], AF.Identity,
                                         bias=sb3t[:, m:m + 1], scale=1.0)
                    nc.vector.tensor_mul(ht[:, m, :], hs[:], h3[:])
                if not skip_bias2:
                    cwb = cp.tile([E + 1, 512], F32R, tag="cwb")
                    nc.sync.dma_start(out=cwb[:], in_=cwT1.ap()[:, n0:n0 + 512])
                for c in range(4):
                    t0 = n0 + c * P
                    yt = cp.tile([P, DIM], F32, tag="yt")
                    for h in range(2):
                        psz = pp2.tile([P, 512], F32, tag="psz")
                        for k in range(KS):
                            w = s2a if k < 4 else s2b
                            nc.tensor.matmul(out=psz[:, :],
                                             lhsT=ht[:, k, c * P:(c + 1) * P],
                                             rhs=w[:, k % 4, h * 512:(h + 1) * 512],
                                             start=(k == 0),
                                             stop=(skip_bias2 and k == KS - 1))
                        if not skip_bias2:
                            nc.tensor.matmul(out=psz[:, :], lhsT=cwb[:, c * P:(c + 1) * P],
                                             rhs=b2t[:, h * 512:(h + 1) * 512],
                                             start=False, stop=True)
                        nc.scalar.copy(yt[:, h * 512:(h + 1) * 512], psz[:, :])
                    for k in range(4):
                        zt = zp.tile([P, DIM], F16, tag="zc")
                        (nc.sync if k % 2 == 0 else nc.scalar).dma_start(out=zt[:], in_=zbuf.ap()[k * T + t0:k * T + t0 + P, :])
                        nc.vector.tensor_add(yt[:], yt[:], zt[:])
                    (nc.sync if c % 2 == 0 else nc.scalar).dma_start(out=y.ap()[t0:t0 + P, :], in_=yt[:])
    return nc


def _host_route(cw, T):
    """From dense combine weights cw[T, E] build (per-core) routing lists.
    Returns tokens[e] (np arrays), weights[e], slot_of_pair[e]."""
    nz = cw > 0.0
    counts = nz.sum(1)
    toks, wts, slots = [], [], []
    slot_ctr = np.zeros(T, np.int64)
    # tokens with more than TOPK positives (ties): keep top TOPK by value
    drop = {}
    for t in np.nonzero(counts > TOPK)[0]:
        vals = cw[t]
        order = np.argsort(-vals, kind="stable")
        drop[t] = set(order[TOPK:][vals[order[TOPK:]] > 0].tolist())
    for e in range(E):
        tk = np.nonzero(nz[:, e])[0]
        if drop:
            tk = np.array([t for t in tk if not (t in drop and e in drop[t])], dtype=np.int64)
        toks.append(tk)
        wts.append(cw[tk, e])
        sl = slot_ctr[tk].copy()
        slot_ctr[tk] += 1
        slots.append(sl)
    return toks, wts, slots, slot_ctr


def kernel(x, gw, gb, w1, b1, w3, b3, w2, b2, sw1, sb1, sw3, sb3, sw2, sb2):
    x = np.ascontiguousarray(np.asarray(x, np.float32))
    B, S, _ = x.shape
    T = (B * S) // NCORES
    NB = T // 512
    xs = x.reshape(NCORES, T, DIM)
    xT = np.ascontiguousarray(xs.transpose(0, 2, 1))  # [NCORES, DIM, T]
    xTb = xT.astype(NPBF16)
    gb2d = np.ascontiguousarray(np.asarray(gb, np.float32).reshape(E, 1))

    def tile_x(xTc, dt):
        # [DIM, T] -> [NB, P, KD*512]: block b, partition p, free (k, n)
        return np.ascontiguousarray(
            xTc.reshape(KD, P, NB, 512).transpose(2, 1, 0, 3).reshape(NB, P, KD * 512)
        ).astype(dt)

    # ---- launch 1: gate (fp32) ----
    nc1 = build_gate(T)
    nc1.compile()
    gwd = _tile_k(np.asarray(gw, np.float32))  # [P, KD*E]
    in_maps = [{"xTt": tile_x(xT[c], np.float32), "gwd": gwd, "gb": gb2d}
               for c in range(NCORES)]
    res1 = run_bass_kernel_spmd(nc1, in_maps, core_ids=list(range(NCORES)))
    cw_all = np.concatenate([res1.results[c]["cw"] for c in range(NCORES)], 0)  # [B*S, E]

    # ---- host: rebalance token->core assignment (pure data movement) so
    # per-(core, expert) token counts are near-even; shrinks the shared
    # max-over-cores segment plan the device pads to.
    Tall = cw_all.shape[0]
    topi = np.argsort(-cw_all, kind="stable", axis=1)[:, :TOPK]  # >0 entries lead
    cnt2 = np.zeros((NCORES, E), np.int64)
    cap = np.full(NCORES, T, np.int64)
    totals = np.bincount(topi.ravel(), minlength=E)
    target = (totals + NCORES - 1) // NCORES
    perm = [[] for _ in range(NCORES)]
    rng_order = np.random.RandomState(0).permutation(Tall)
    for t in rng_order:
        es = topi[t]
        score = cnt2[:, es].sum(1) * 8 + (T - cap)
        score[cap == 0] = 1 << 60
        c = int(np.argmin(score))
        perm[c].append(t)
        cnt2[c, es] += 1
        cap[c] -= 1
    perm = [np.array(p, np.int64) for p in perm]
    xflat = x.reshape(B * S, DIM)
    xT = np.stack([np.ascontiguousarray(xflat[perm[c]].T) for c in range(NCORES)])
    xTb = xT.astype(NPBF16)
    cws = [np.ascontiguousarray(cw_all[perm[c]]) for c in range(NCORES)]

    # ---- host: build routing metadata (data movement only) ----
    routed = [_host_route(cws[c], T) for c in range(NCORES)]
    cnt = np.array([[len(routed[c][0][e]) for e in range(E)] for c in range(NCORES)])
    seg_len = cnt.max(0)  # shared static plan across cores
    seg_len = ((seg_len + 3) // 4) * 4  # even moving dim for the matmuls
    seg_start = np.concatenate([[0], np.cumsum(seg_len)]).astype(int)
    Lsum = int(seg_len.sum())
    DUMMY = 4 * T
    blocks = [(e, n0, nlen) for e in range(E) if seg_len[e] > 0
              for (n0, nlen) in _chunks(int(seg_len[e]), 512)]
    nblk = len(blocks)
    nfull = sum(1 for (_, _, nlen) in blocks if nlen == 512)
    tmax = max([nlen for (_, _, nlen) in blocks if nlen < 512], default=4)

    xgts, xgls, metas, cwT1s = [], [], [], []
    for c in range(NCORES):
        toks, wts, slots, slot_ctr = routed[c]
        xg = np.zeros((DIM, Lsum), NPBF16)
        pwv = np.zeros((Lsum,), np.float32)
        sov = np.full((Lsum,), DUMMY, np.int32)
        pad_list = []
        for e in range(E):
            s0 = seg_start[e]
            n = len(toks[e])
            if n:
                xg[:, s0:s0 + n] = xTb[c][:, toks[e]]
                pwv[s0:s0 + n] = wts[e]
                sov[s0:s0 + n] = (slots[e] * T + toks[e]).astype(np.int32)
            pad_list.extend(range(s0 + n, s0 + int(seg_len[e])))
        # route missing (token, slot) pairs (from dropped ties) to padding pairs,
        # which compute exact zeros -> correct "no contribution" rows.
        miss = [(t, s) for t in np.nonzero(slot_ctr < TOPK)[0]
                for s in range(int(slot_ctr[t]), TOPK)]
        assert len(miss) <= len(pad_list), "not enough padding slots"
        for (t, s), j in zip(miss, pad_list):
            sov[j] = np.int32(s * T + t)
        # pre-tiled xg: full blocks [nfull, P, KD*512], tails [E, P, KD*tmax]
        xgt = np.zeros((max(nfull, 1), P, KD * 512), NPBF16)
        xgl = np.zeros((E, P, KD * max(tmax, 4)), NPBF16)
        jf = 0
        for (e, n0, nlen) in blocks:
            g0 = seg_start[e] + n0
            blk = xg[:, g0:g0 + nlen].reshape(KD, P, nlen).transpose(1, 0, 2)  # [P, KD, nlen]
            if nlen == 512:
                xgt[jf] = blk.reshape(P, KD * 512)
                jf += 1
            else:
                xgl[e].reshape(P, KD, max(tmax, 4))[:, :, :nlen] = blk
        # merged per-block metadata [P, nblk, 8]: [:,:,0:4]=pw bits, [:,:,4:8]=soff
        mt = np.zeros((P, nblk, 8), np.int32)
        mt[:, :, 4:8] = DUMMY
        for j, (e, n0, nlen) in enumerate(blocks):
            g0 = seg_start[e] + n0
            idx = np.arange(nlen)
            mt[idx % P, j, idx // P] = pwv[g0:g0 + nlen].view(np.int32)
            mt[idx % P, j, 4 + idx // P] = sov[g0:g0 + nlen]
        xgts.append(xgt)
        xgls.append(xgl)
        metas.append(np.ascontiguousarray(mt.reshape(P, nblk * 8)))
        cwT1s.append(np.ascontiguousarray(
            np.concatenate([cws[c].T, np.ones((1, T), np.float32)], 0)))

    b2a = np.ascontiguousarray(np.concatenate(
        [np.asarray(b2, np.float32), np.asarray(sb2, np.float32).reshape(1, DIM)], 0))

    # ---- launch 2: main (bf16 matmuls, fp32 accumulate) ----
    skip_bias2 = not b2a.any()
    nc2 = build_main(T, seg_len, nblk, nfull, tmax, skip_bias2=skip_bias2)
    nc2.compile()
    asf32 = lambda a: np.asarray(a, np.float32)
    w1d = np.stack([_tile_k(asf32(w1)[e]) for e in range(E)]).astype(NPBF16)
    w3d = np.stack([_tile_k(asf32(w3)[e]) for e in range(E)]).astype(NPBF16)
    w2d = np.stack([_tile_k(asf32(w2)[e]) for e in range(E)]).astype(NPBF16)
    sw1d = np.stack([_tile_k(asf32(sw1)[h * 512:(h + 1) * 512]) for h in range(2)]).astype(NPBF16)
    sw3d = np.stack([_tile_k(asf32(sw3)[h * 512:(h + 1) * 512]) for h in range(2)]).astype(NPBF16)
    sw2d = np.stack([_tile_k(asf32(sw2)[h * 512:(h + 1) * 512]) for h in range(2)]).astype(NPBF16)
    b1dd = np.ascontiguousarray(asf32(b1).reshape(E, KI, P).transpose(2, 0, 1).reshape(P, E * KI))
    b3dd = np.ascontiguousarray(asf32(b3).reshape(E, KI, P).transpose(2, 0, 1).reshape(P, E * KI))
    sb1d = np.ascontiguousarray(asf32(sb1).reshape(KS, P).T)
    sb3d = np.ascontiguousarray(asf32(sb3).reshape(KS, P).T)
    in_maps = [{
        "xTt": tile_x(xTb[c], NPBF16), "xgt": xgts[c], "xgl": xgls[c],
        "meta": metas[c], "cwT1": cwT1s[c], "b2a": b2a,
        "w1": w1d, "w3": w3d, "w2": w2d, "b1d": b1dd, "b3d": b3dd,
        "sw1": sw1d, "sw3": sw3d, "sw2": sw2d, "sb1": sb1d, "sb3": sb3d,
    } for c in range(NCORES)]
    res2 = run_bass_kernel_spmd(nc2, in_maps, core_ids=list(range(NCORES)))
    yfull = np.empty((B * S, DIM), np.float32)
    for c in range(NCORES):
        yfull[perm[c]] = res2.results[c]["y"]
    return yfull.reshape(B, S, DIM)


# revision 22
# speedup vs baseline: 1.0291x; 1.0026x over previous
"""MoE (DeepSeek-style gate + 32 routed SwiGLU experts + shared expert) on 8 trn2 cores.

Strategy: data-parallel over tokens (batch dim 8 -> 1 slab per core), expert
weights replicated.  Two device launches per call:

  1. gate kernel: computes dense combine-weights cw[T, E] (softmax + grouped
     top-k routing) on device.  Plain fp32 matmuls (routing decisions are
     tie-sensitive; bf16 scores flip ~1% of token expert sets, fp32r flips
     ~10/32k - both blow the error budget).  Weights-stationary orientation:
     lhsT=gw [128,32], rhs=x [128,512], scores land transposed [E, tokens].
  2. main kernel: per expert, gathers its tokens (host builds the gather
     layout from cw - pure data movement), runs the SwiGLU expert MLP in
     bf16 (fp32 PSUM accumulate), scales by routing weight, scatters rows
     into a slot buffer, then combines slots + shared-expert output +
     (b2/sb2 via a small cw @ [b2;sb2] matmul) into y.

All tensors the device streams at full rate are PRE-TILED on the host into
their exact SBUF layout so every DMA line is >=2 KB contiguous per partition
(1 KB-line layouts measured ~130 GB/s vs ~358 peak and stalled the PE).
All arithmetic happens on device; the host only reshapes/permutes/casts data.
"""

import sys

sys.path.insert(0, "/opt/trn_rl_repo")

import numpy as np
import ml_dtypes

import concourse.bacc as bacc
import concourse.mybir as mybir
import concourse.tile as tile
from concourse import bass
from concourse.bass_utils import run_bass_kernel_spmd
from concourse.masks import make_identity

NCORES = 8
DIM = 1024
INTER = 512
E = 32
TOPK = 4
GROUPS = 8
TOPK_G = 4
SINTER = 1024
P = 128
KD = DIM // P     # 8 k-tiles over dim
KI = INTER // P   # 4 k-tiles over inter
KS = SINTER // P  # 8 k-tiles over shared inter

F32 = mybir.dt.float32
F32R = mybir.dt.float32r
BF16 = mybir.dt.bfloat16
F16 = mybir.dt.float16
I32 = mybir.dt.int32
AF = mybir.ActivationFunctionType
OP = mybir.AluOpType
AX = mybir.AxisListType
NPBF16 = ml_dtypes.bfloat16


def _chunks(total, size):
    out = []
    off = 0
    while off < total:
        out.append((off, min(size, total - off)))
        off += size
    return out


def _tile_k(w, p=P):
    """[K, N] -> [p, (K//p)*N]: row k*p+q lands at partition q, free k*N..k*N+N."""
    K, N = w.shape
    return np.ascontiguousarray(
        w.reshape(K // p, p, N).transpose(1, 0, 2).reshape(p, (K // p) * N))


def build_gate(T):
    nc = bacc.Bacc("TRN2", target_bir_lowering=False)
    NB = T // 512
    xTt = nc.dram_tensor("xTt", [NB, P, KD * 512], F32, kind="ExternalInput")
    gwd = nc.dram_tensor("gwd", [P, KD * E], F32, kind="ExternalInput")
    gb = nc.dram_tensor("gb", [E, 1], F32, kind="ExternalInput")
    cw = nc.dram_tensor("cw", [T, E], F32, kind="ExternalOutput")
    with tile.TileContext(nc) as tc:
        with tc.tile_pool(name="cst", bufs=1) as cst, \
             tc.tile_pool(name="xp", bufs=3) as xp, \
             tc.tile_pool(name="sb", bufs=2) as sb, \
             tc.tile_pool(name="ps", bufs=2, space="PSUM") as ps, \
             tc.tile_pool(name="pt", bufs=4, space="PSUM") as pt:
            gwt = cst.tile([P, KD, E], F32)
            nc.sync.dma_start(out=gwt[:], in_=gwd.ap().rearrange("p (k e) -> p k e", k=KD))
            gbt = cst.tile([E, 1], F32)
            nc.sync.dma_start(out=gbt[:], in_=gb.ap())
            ident = cst.tile([P, P], F32)
            make_identity(nc, ident[:])
            for b in range(NB):
                n0 = b * 512
                xt = xp.tile([P, KD, 512], F32, tag="xt")
                nc.sync.dma_start(out=xt[:], in_=xTt.ap()[b].rearrange("p (k n) -> p k n", k=KD))
                # scores transposed: [E, 512] = gw.T @ x
                s = ps.tile([E, 512], F32, tag="s")
                for k in range(KD):
                    nc.tensor.matmul(out=s[:], lhsT=gwt[:, k, :], rhs=xt[:, k, :],
                                     start=(k == 0), stop=(k == KD - 1))
                sc = sb.tile([E, 512], F32, tag="sc")
                nc.scalar.activation(sc[:], s[:], AF.Identity, bias=gbt[:, 0:1], scale=1.0)
                # PE-transpose back to [tokens, E], 4 chunks of 128
                s4 = sb.tile([P, 4, E], F32, tag="s4")
                for c in range(4):
                    pst = pt.tile([P, E], F32, tag="pst")
                    nc.tensor.transpose(out=pst[:], in_=sc[:, c * P:(c + 1) * P],
                                        identity=ident[:E, :E])
                    nc.scalar.copy(s4[:, c, :], pst[:])
                # batched softmax over the innermost 32
                negmax = sb.tile([P, 4], F32, tag="negmax")
                nc.vector.tensor_reduce(out=negmax[:], in_=s4[:], op=OP.max, axis=AX.X,
                                        negate=True)
                sh = sb.tile([P, 4, E], F32, tag="sh")
                nc.vector.tensor_tensor(out=sh[:], in0=s4[:],
                                        in1=negmax[:].unsqueeze(2).to_broadcast([P, 4, E]),
                                        op=OP.add)
                et = sb.tile([P, 4, E], F32, tag="et")
                nc.scalar.activation(et[:], sh[:], AF.Exp)
                ssum = sb.tile([P, 4], F32, tag="ssum")
                nc.vector.reduce_sum(out=ssum[:], in_=et[:], axis=AX.X)
                rsum = sb.tile([P, 4], F32, tag="rsum")
                nc.vector.reciprocal(rsum[:], ssum[:])
                pr = sb.tile([P, 4, E], F32, tag="pr")
                nc.vector.tensor_tensor(out=pr[:], in0=et[:],
                                        in1=rsum[:].unsqueeze(2).to_broadcast([P, 4, E]),
                                        op=OP.mult)
                # group scores: sum of top-2 within each group of 4
                # top2sum(a,b,c,d) = max(a+b, c+d, max(a,b)+max(c,d))
                g = pr[:].rearrange("p c (g f) -> p c g f", f=4)
                ga = sb.tile([P, 4, GROUPS], F32, tag="ga")
                gbv = sb.tile([P, 4, GROUPS], F32, tag="gbv")
                m1 = sb.tile([P, 4, GROUPS], F32, tag="m1")
                gsc = sb.tile([P, 4, GROUPS], F32, tag="gsc")
                nc.vector.tensor_add(ga[:], g[:, :, :, 0], g[:, :, :, 1])
                nc.vector.tensor_add(gbv[:], g[:, :, :, 2], g[:, :, :, 3])
                nc.vector.tensor_tensor(out=m1[:], in0=g[:, :, :, 0], in1=g[:, :, :, 1],
                                        op=OP.max)
                nc.vector.tensor_tensor(out=gsc[:], in0=g[:, :, :, 2], in1=g[:, :, :, 3],
                                        op=OP.max)
                nc.vector.tensor_add(m1[:], m1[:], gsc[:])
                nc.vector.tensor_tensor(out=ga[:], in0=ga[:], in1=gbv[:], op=OP.max)
                nc.vector.tensor_tensor(out=gsc[:], in0=ga[:], in1=m1[:], op=OP.max)
                # keep top-4 groups, then top-4 experts within kept groups
                keep = sb.tile([P, 4, GROUPS], F32, tag="keep")
                for c in range(4):
                    srt = sb.tile([P, 8], F32, tag="srt")
                    nc.vector.max(srt[:], gsc[:, c, :])
                    nc.vector.tensor_scalar(keep[:, c, :], gsc[:, c, :], srt[:, 3:4],
                                            None, op0=OP.is_ge)
                masked = sb.tile([P, 4, E], F32, tag="masked")
                nc.vector.tensor_tensor(
                    out=masked[:].rearrange("p c (g f) -> p c g f", f=4),
                    in0=g,
                    in1=keep[:].unsqueeze(3).to_broadcast([P, 4, GROUPS, 4]),
                    op=OP.mult,
                )
                cwt = sb.tile([P, 4, E], F32, tag="cwt")
                for c in range(4):
                    srt2 = sb.tile([P, 8], F32, tag="srt2")
                    nc.vector.max(srt2[:], masked[:, c, :])
                    nc.vector.tensor_scalar(cwt[:, c, :], masked[:, c, :], srt2[:, 3:4],
                                            None, op0=OP.is_ge)
                nc.vector.tensor_mul(cwt[:], cwt[:], masked[:])
                nc.sync.dma_start(
                    out=cw.ap()[n0:n0 + 512, :].rearrange("(c p) e -> p c e", p=P),
                    in_=cwt[:])
    return nc


def build_main(T, seg_len, nblk, nfull, tmax, skip_bias2=False):
    """seg_len[e]: padded token count for expert e (same across cores).
    zbuf rows: slot k of token t at k*T+t, dummy rows (padding pairs) at 4*T.
    nblk: total 512-token blocks (incl. tails), nfull: full blocks only,
    tmax: padded max tail length."""
    nc = bacc.Bacc("TRN2", target_bir_lowering=False)
    NB = T // 512
    xTt = nc.dram_tensor("xTt", [NB, P, KD * 512], BF16, kind="ExternalInput")
    xgt = nc.dram_tensor("xgt", [max(nfull, 1), P, KD * 512], BF16, kind="ExternalInput")
    xgl = nc.dram_tensor("xgl", [E, P, KD * max(tmax, 4)], BF16, kind="ExternalInput")
    meta = nc.dram_tensor("meta", [P, nblk * 8], I32, kind="ExternalInput")
    cwT1 = nc.dram_tensor("cwT1", [E + 1, T], F32R, kind="ExternalInput")
    b2a = nc.dram_tensor("b2a", [E + 1, DIM], F32R, kind="ExternalInput")
    w1 = nc.dram_tensor("w1", [E, P, KD * INTER], BF16, kind="ExternalInput")
    w3 = nc.dram_tensor("w3", [E, P, KD * INTER], BF16, kind="ExternalInput")
    w2 = nc.dram_tensor("w2", [E, P, KI * DIM], BF16, kind="ExternalInput")
    b1d = nc.dram_tensor("b1d", [P, E * KI], F32, kind="ExternalInput")
    b3d = nc.dram_tensor("b3d", [P, E * KI], F32, kind="ExternalInput")
    sw1 = nc.dram_tensor("sw1", [2, P, 4 * SINTER], BF16, kind="ExternalInput")
    sw3 = nc.dram_tensor("sw3", [2, P, 4 * SINTER], BF16, kind="ExternalInput")
    sw2 = nc.dram_tensor("sw2", [2, P, 4 * DIM], BF16, kind="ExternalInput")
    sb1 = nc.dram_tensor("sb1", [P, KS], F32, kind="ExternalInput")
    sb3 = nc.dram_tensor("sb3", [P, KS], F32, kind="ExternalInput")
    y = nc.dram_tensor("y", [T, DIM], F32, kind="ExternalOutput")
    zbuf = nc.dram_tensor("zbuf", [4 * T + P, DIM], F16)

    from contextlib import ExitStack
    with tile.TileContext(nc) as tc:
        with ExitStack() as ctx:
            cst = ctx.enter_context(tc.tile_pool(name="cst", bufs=1))
            wp = ctx.enter_context(tc.tile_pool(name="wp", bufs=3))
            sp = ctx.enter_context(tc.tile_pool(name="sp", bufs=1))
            xp = ctx.enter_context(tc.tile_pool(name="xp", bufs=3))
            hp = ctx.enter_context(tc.tile_pool(name="hp", bufs=2))
            ep = ctx.enter_context(tc.tile_pool(name="ep", bufs=2))
            zp = ctx.enter_context(tc.tile_pool(name="zp", bufs=2))
            cp = ctx.enter_context(tc.tile_pool(name="cp", bufs=3))
            pp1 = ctx.enter_context(tc.tile_pool(name="pp1", bufs=2, space="PSUM"))
            pp2 = ctx.enter_context(tc.tile_pool(name="pp2", bufs=3, space="PSUM"))

            metat = cst.tile([P, nblk, 8], I32)
            nc.sync.dma_start(out=metat[:], in_=meta.ap().rearrange("p (j m) -> p j m", m=8))
            b1all = cst.tile([P, E, KI], F32)
            nc.sync.dma_start(out=b1all[:], in_=b1d.ap().rearrange("p (e m) -> p e m", m=KI))
            b3all = cst.tile([P, E, KI], F32)
            nc.sync.dma_start(out=b3all[:], in_=b3d.ap().rearrange("p (e m) -> p e m", m=KI))

            def up_proj(xt, w1t, w3t, e, ht, m, nlen):
                """ht[:, m, :nlen] = silu(w1^T x + b1) * (w3^T x + b3) for inter tile m."""
                ps1 = pp1.tile([P, 512], F32, tag="ps1")
                for k in range(KD):
                    nc.tensor.matmul(out=ps1[:, :nlen], lhsT=w1t[:, k, m * P:(m + 1) * P],
                                     rhs=xt[:, k, :nlen], start=(k == 0), stop=(k == KD - 1))
                ps3 = pp1.tile([P, 512], F32, tag="ps3")
                for k in range(KD):
                    nc.tensor.matmul(out=ps3[:, :nlen], lhsT=w3t[:, k, m * P:(m + 1) * P],
                                     rhs=xt[:, k, :nlen], start=(k == 0), stop=(k == KD - 1))
                hs = ep.tile([P, 512], F32, tag="hs")
                nc.scalar.activation(hs[:, :nlen], ps1[:, :nlen], AF.Silu,
                                     bias=b1all[:, e, m:m + 1], scale=1.0)
                h3 = ep.tile([P, 512], F32, tag="h3")
                nc.scalar.activation(h3[:, :nlen], ps3[:, :nlen], AF.Identity,
                                     bias=b3all[:, e, m:m + 1], scale=1.0)
                nc.vector.tensor_mul(ht[:, m, :nlen], hs[:, :nlen], h3[:, :nlen])

            # ---------------- phase A: routed experts ----------------
            order = [e for e in range(E) if seg_len[e] > 0]
            shared_tiles = {}

            def load_shared():
                tiles = {}
                for name, src in (("s1", sw1), ("s3", sw3), ("s2", sw2)):
                    for half in range(2):
                        t = sp.tile([P, 4, SINTER], BF16, tag=f"{name}{half}")
                        ap = src.ap()[half].rearrange("p (k i) -> p k i", k=4)
                        (nc.gpsimd if half == 0 else nc.scalar).dma_start(out=t[:], in_=ap)
                        tiles[f"{name}{half}"] = t
                shared_tiles.update(tiles)

            jblk = 0
            jfull = 0
            for ei, e in enumerate(order):
                e = int(e)
                if ei == len(order) - 2:
                    # prefetch shared-expert weights during the tail of phase A
                    load_shared()
                w1t = wp.tile([P, KD, INTER], BF16, tag="w1e")
                w3t = wp.tile([P, KD, INTER], BF16, tag="w3e")
                w1ap = w1.ap()[e].rearrange("p (k i) -> p k i", k=KD)
                w3ap = w3.ap()[e].rearrange("p (k i) -> p k i", k=KD)
                if ei == 0:
                    # fine-grained first loads: PE can start on k-slice 0 asap
                    for k in range(KD):
                        nc.scalar.dma_start(out=w1t[:, k, :], in_=w1ap[:, k, :])
                    for k in range(KD):
                        nc.gpsimd.dma_start(out=w3t[:, k, :], in_=w3ap[:, k, :])
                else:
                    nc.scalar.dma_start(out=w1t[:], in_=w1ap)
                    nc.gpsimd.dma_start(out=w3t[:], in_=w3ap)
                w2t = wp.tile([P, KI, DIM], BF16, tag="w2e")
                w2ap = w2.ap()[e].rearrange("p (k d) -> p k d", k=KI)
                nc.scalar.dma_start(out=w2t[:], in_=w2ap)
                for (n0, nlen) in _chunks(int(seg_len[e]), 512):
                    if nlen == 512:
                        xt = xp.tile([P, KD, 512], BF16, tag="xg")
                        xap = xgt.ap()[jfull].rearrange("p (k n) -> p k n", k=KD)
                        if ei == 0 and n0 == 0:
                            for k in range(KD):
                                nc.sync.dma_start(out=xt[:, k, :], in_=xap[:, k, :])
                        else:
                            nc.sync.dma_start(out=xt[:], in_=xap)
                        jfull += 1
                    else:
                        xt = xp.tile([P, KD, 512], BF16, tag="xg")
                        nc.sync.dma_start(
                            out=xt[:, :, :nlen],
                            in_=xgl.ap()[e].rearrange("p (k n) -> p k n", k=KD)[:, :, :nlen])
                    j = jblk
                    jblk += 1
                    nch = (nlen + P - 1) // P
                    ht = hp.tile([P, KS, 512], BF16, tag="ht")
                    for m in range(KI):
                        up_proj(xt, w1t, w3t, e, ht, m, nlen)
                    for c in range(nch):
                        cl = min(P, nlen - c * P)
                        zt = zp.tile([P, DIM], F16, tag="zt")
                        for h in range(2):
                            psz = pp2.tile([P, 512], F32, tag="psz")
                            for k in range(KI):
                                nc.tensor.matmul(out=psz[:cl, :],
                                                 lhsT=ht[:, k, c * P:c * P + cl],
                                                 rhs=w2t[:, k, h * 512:(h + 1) * 512],
                                                 start=(k == 0), stop=(k == KI - 1))
                            nc.scalar.activation(
                                zt[:cl, h * 512:(h + 1) * 512], psz[:cl, :],
                                AF.Copy, scale=metat[:cl, j, c:c + 1].bitcast(F32))
                        nc.gpsimd.indirect_dma_start(
                            out=zbuf.ap(),
                            out_offset=bass.IndirectOffsetOnAxis(
                                ap=metat[:cl, j, 4 + c:5 + c], axis=0),
                            in_=zt[:cl, :],
                            in_offset=None,
                        )

            # ------- phase B+C fused: shared expert + combine per 512 tokens -------
            if not shared_tiles:
                load_shared()
            s1a, s1b = shared_tiles["s10"], shared_tiles["s11"]
            s3a, s3b = shared_tiles["s30"], shared_tiles["s31"]
            s2a, s2b = shared_tiles["s20"], shared_tiles["s21"]
            sb1t = cst.tile([P, KS], F32)
            nc.sync.dma_start(out=sb1t[:], in_=sb1.ap())
            sb3t = cst.tile([P, KS], F32)
            nc.sync.dma_start(out=sb3t[:], in_=sb3.ap())
            if not skip_bias2:
                b2t = cst.tile([E + 1, DIM], F32R)
                nc.sync.dma_start(out=b2t[:], in_=b2a.ap())

            for b in range(NB):
                n0 = b * 512
                xt = xp.tile([P, KD, 512], BF16, tag="xg")
                xap = xTt.ap()[b].rearrange("p (k n) -> p k n", k=KD)
                nc.sync.dma_start(out=xt[:, 0:4, :], in_=xap[:, 0:4, :])
                nc.scalar.dma_start(out=xt[:, 4:8, :], in_=xap[:, 4:8, :])
                ht = hp.tile([P, KS, 512], BF16, tag="ht")
                for m in range(KS):
                    ps1 = pp1.tile([P, 512], F32, tag="ps1")
                    for k in range(KD):
                        w = s1a if k < 4 else s1b
                        nc.tensor.matmul(out=ps1[:], lhsT=w[:, k % 4, m * P:(m + 1) * P],
                                         rhs=xt[:, k, :], start=(k == 0), stop=(k == KD - 1))
                    ps3 = pp1.tile([P, 512], F32, tag="ps3")
                    for k in range(KD):
                        w = s3a if k < 4 else s3b
                        nc.tensor.matmul(out=ps3[:], lhsT=w[:, k % 4, m * P:(m + 1) * P],
                                         rhs=xt[:, k, :], start=(k == 0), stop=(k == KD - 1))
                    hs = ep.tile([P, 512], F32, tag="hs")
                    nc.scalar.activation(hs[:], ps1[:], AF.Silu,
                                         bias=sb1t[:, m:m + 1], scale=1.0)
                    h3 = ep.tile([P, 512], F32, tag="h3")
                    nc.scalar.activation(h3[:], ps3[:], AF.Identity,
                                         bias=sb3t[:, m:m + 1], scale=1.0)
                    nc.vector.tensor_mul(ht[:, m, :], hs[:], h3[:])
                if not skip_bias2:
                    cwb = cp.tile([E + 1, 512], F32R, tag="cwb")
                    nc.sync.dma_start(out=cwb[:], in_=cwT1.ap()[:, n0:n0 + 512])
                for c in range(4):
                    t0 = n0 + c * P
                    yt = cp.tile([P, DIM], F32, tag="yt")
                    for h in range(2):
                        psz = pp2.tile([P, 512], F32, tag="psz")
                        for k in range(KS):
                            w = s2a if k < 4 else s2b
                            nc.tensor.matmul(out=psz[:, :],
                                             lhsT=ht[:, k, c * P:(c + 1) * P],
                                             rhs=w[:, k % 4, h * 512:(h + 1) * 512],
                                             start=(k == 0),
                                             stop=(skip_bias2 and k == KS - 1))
                        if not skip_bias2:
                            nc.tensor.matmul(out=psz[:, :], lhsT=cwb[:, c * P:(c + 1) * P],
                                             rhs=b2t[:, h * 512:(h + 1) * 512],
                                             start=False, stop=True)
                        nc.scalar.copy(yt[:, h * 512:(h + 1) * 512], psz[:, :])
                    for k in range(4):
                        zt = zp.tile([P, DIM], F16, tag="zc")
                        (nc.sync if k % 2 == 0 else nc.scalar).dma_start(out=zt[:], in_=zbuf.ap()[k * T + t0:k * T + t0 + P, :])
                        nc.vector.tensor_add(yt[:], yt[:], zt[:])
                    (nc.sync if c % 2 == 0 else nc.scalar).dma_start(out=y.ap()[t0:t0 + P, :], in_=yt[:])
    return nc


def _host_route(cw, T):
    """From dense combine weights cw[T, E] build (per-core) routing lists.
    Returns tokens[e] (np arrays), weights[e], slot_of_pair[e]."""
    nz = cw > 0.0
    counts = nz.sum(1)
    toks, wts, slots = [], [], []
    slot_ctr = np.zeros(T, np.int64)
    # tokens with more than TOPK positives (ties): keep top TOPK by value
    drop = {}
    for t in np.nonzero(counts > TOPK)[0]:
        vals = cw[t]
        order = np.argsort(-vals, kind="stable")
        drop[t] = set(order[TOPK:][vals[order[TOPK:]] > 0].tolist())
    for e in range(E):
        tk = np.nonzero(nz[:, e])[0]
        if drop:
            tk = np.array([t for t in tk if not (t in drop and e in drop[t])], dtype=np.int64)
        toks.append(tk)
        wts.append(cw[tk, e])
        sl = slot_ctr[tk].copy()
        slot_ctr[tk] += 1
        slots.append(sl)
    return toks, wts, slots, slot_ctr


def kernel(x, gw, gb, w1, b1, w3, b3, w2, b2, sw1, sb1, sw3, sb3, sw2, sb2):
    x = np.ascontiguousarray(np.asarray(x, np.float32))
    B, S, _ = x.shape
    T = (B * S) // NCORES
    NB = T // 512
    xs = x.reshape(NCORES, T, DIM)
    xT = np.ascontiguousarray(xs.transpose(0, 2, 1))  # [NCORES, DIM, T]
    xTb = xT.astype(NPBF16)
    gb2d = np.ascontiguousarray(np.asarray(gb, np.float32).reshape(E, 1))

    def tile_x(xTc, dt):
        # [DIM, T] -> [NB, P, KD*512]: block b, partition p, free (k, n)
        return np.ascontiguousarray(
            xTc.reshape(KD, P, NB, 512).transpose(2, 1, 0, 3).reshape(NB, P, KD * 512)
        ).astype(dt)

    # ---- launch 1: gate (fp32) ----
    nc1 = build_gate(T)
    nc1.compile()
    gwd = _tile_k(np.asarray(gw, np.float32))  # [P, KD*E]
    in_maps = [{"xTt": tile_x(xT[c], np.float32), "gwd": gwd, "gb": gb2d}
               for c in range(NCORES)]
    res1 = run_bass_kernel_spmd(nc1, in_maps, core_ids=list(range(NCORES)))
    cw_all = np.concatenate([res1.results[c]["cw"] for c in range(NCORES)], 0)  # [B*S, E]

    # ---- host: rebalance token->core assignment (pure data movement) so
    # per-(core, expert) token counts are near-even; shrinks the shared
    # max-over-cores segment plan the device pads to.
    Tall = cw_all.shape[0]
    topi = np.argsort(-cw_all, kind="stable", axis=1)[:, :TOPK]  # >0 entries lead
    cnt2 = np.zeros((NCORES, E), np.int64)
    cap = np.full(NCORES, T, np.int64)
    totals = np.bincount(topi.ravel(), minlength=E)
    target = (totals + NCORES - 1) // NCORES
    perm = [[] for _ in range(NCORES)]
    rng_order = np.random.RandomState(0).permutation(Tall)
    for t in rng_order:
        es = topi[t]
        score = cnt2[:, es].sum(1) * 8 + (T - cap)
        score[cap == 0] = 1 << 60
        c = int(np.argmin(score))
        perm[c].append(t)
        cnt2[c, es] += 1
        cap[c] -= 1
    perm = [np.array(p, np.int64) for p in perm]
    xflat = x.reshape(B * S, DIM)
    xT = np.stack([np.ascontiguousarray(xflat[perm[c]].T) for c in range(NCORES)])
    xTb = xT.astype(NPBF16)
    cws = [np.ascontiguousarray(cw_all[perm[c]]) for c in range(NCORES)]

    # ---- host: build routing metadata (data movement only) ----
    routed = [_host_route(cws[c], T) for c in range(NCORES)]
    cnt = np.array([[len(routed[c][0][e]) for e in range(E)] for c in range(NCORES)])
    seg_len = cnt.max(0)  # shared static plan across cores
    seg_len = ((seg_len + 3) // 4) * 4  # even moving dim for the matmuls
    seg_start = np.concatenate([[0], np.cumsum(seg_len)]).astype(int)
    Lsum = int(seg_len.sum())
    DUMMY = 4 * T
    blocks = [(e, n0, nlen) for e in range(E) if seg_len[e] > 0
              for (n0, nlen) in _chunks(int(seg_len[e]), 512)]
    nblk = len(blocks)
    nfull = sum(1 for (_, _, nlen) in blocks if nlen == 512)
    tmax = max([nlen for (_, _, nlen) in blocks if nlen < 512], default=4)

    xgts, xgls, metas, cwT1s = [], [], [], []
    for c in range(NCORES):
        toks, wts, slots, slot_ctr = routed[c]
        xg = np.zeros((DIM, Lsum), NPBF16)
        pwv = np.zeros((Lsum,), np.float32)
        sov = np.full((Lsum,), DUMMY, np.int32)
        pad_list = []
        for e in range(E):
            s0 = seg_start[e]
            n = len(toks[e])
            if n:
                xg[:, s0:s0 + n] = xTb[c][:, toks[e]]
                pwv[s0:s0 + n] = wts[e]
                sov[s0:s0 + n] = (slots[e] * T + toks[e]).astype(np.int32)
            pad_list.extend(range(s0 + n, s0 + int(seg_len[e])))
        # route missing (token, slot) pairs (from dropped ties) to padding pairs,
        # which compute exact zeros -> correct "no contribution" rows.
        miss = [(t, s) for t in np.nonzero(slot_ctr < TOPK)[0]
                for s in range(int(slot_ctr[t]), TOPK)]
        assert len(miss) <= len(pad_list), "not enough padding slots"
        for (t, s), j in zip(miss, pad_list):
            sov[j] = np.int32(s * T + t)
        # pre-tiled xg: full blocks [nfull, P, KD*512], tails [E, P, KD*tmax]
        xgt = np.zeros((max(nfull, 1), P, KD * 512), NPBF16)
        xgl = np.zeros((E, P, KD * max(tmax, 4)), NPBF16)
        jf = 0
        for (e, n0, nlen) in blocks:
            g0 = seg_start[e] + n0
            blk = xg[:, g0:g0 + nlen].reshape(KD, P, nlen).transpose(1, 0, 2)  # [P, KD, nlen]
            if nlen == 512:
                xgt[jf] = blk.reshape(P, KD * 512)
                jf += 1
            else:
                xgl[e].reshape(P, KD, max(tmax, 4))[:, :, :nlen] = blk
        # merged per-block metadata [P, nblk, 8]: [:,:,0:4]=pw bits, [:,:,4:8]=soff
        mt = np.zeros((P, nblk, 8), np.int32)
        mt[:, :, 4:8] = DUMMY
        for j, (e, n0, nlen) in enumerate(blocks):
            g0 = seg_start[e] + n0
            idx = np.arange(nlen)
            mt[idx % P, j, idx // P] = pwv[g0:g0 + nlen].view(np.int32)
            mt[idx % P, j, 4 + idx // P] = sov[g0:g0 + nlen]
        xgts.append(xgt)
        xgls.append(xgl)
        metas.append(np.ascontiguousarray(mt.reshape(P, nblk * 8)))
        cwT1s.append(np.ascontiguousarray(
            np.concatenate([cws[c].T, np.ones((1, T), np.float32)], 0)))

    b2a = np.ascontiguousarray(np.concatenate(
        [np.asarray(b2, np.float32), np.asarray(sb2, np.float32).reshape(1, DIM)], 0))

    # ---- launch 2: main (bf16 matmuls, fp32 accumulate) ----
    skip_bias2 = not b2a.any()
    nc2 = build_main(T, seg_len, nblk, nfull, tmax, skip_bias2=skip_bias2)
    nc2.compile()
    asf32 = lambda a: np.asarray(a, np.float32)
    w1d = np.stack([_tile_k(asf32(w1)[e]) for e in range(E)]).astype(NPBF16)
    w3d = np.stack([_tile_k(asf32(w3)[e]) for e in range(E)]).astype(NPBF16)
    w2d = np.stack([_tile_k(asf32(w2)[e]) for e in range(E)]).astype(NPBF16)
    sw1d = np.stack([_tile_k(asf32(sw1)[h * 512:(h + 1) * 512]) for h in range(2)]).astype(NPBF16)
    sw3d = np.stack([_tile_k(asf32(sw3)[h * 512:(h + 1) * 512]) for h in range(2)]).astype(NPBF16)
    sw2d = np.stack([_tile_k(asf32(sw2)[h * 512:(h + 1) * 512]) for h in range(2)]).astype(NPBF16)
    b1dd = np.ascontiguousarray(asf32(b1).reshape(E, KI, P).transpose(2, 0, 1).reshape(P, E * KI))
    b3dd = np.ascontiguousarray(asf32(b3).reshape(E, KI, P).transpose(2, 0, 1).reshape(P, E * KI))
    sb1d = np.ascontiguousarray(asf32(sb1).reshape(KS, P).T)
    sb3d = np.ascontiguousarray(asf32(sb3).reshape(KS, P).T)
    in_maps = [{
        "xTt": tile_x(xTb[c], NPBF16), "xgt": xgts[c], "xgl": xgls[c],
        "meta": metas[c], "cwT1": cwT1s[c], "b2a": b2a,
        "w1": w1d, "w3": w3d, "w2": w2d, "b1d": b1dd, "b3d": b3dd,
        "sw1": sw1d, "sw3": sw3d, "sw2": sw2d, "sb1": sb1d, "sb3": sb3d,
    } for c in range(NCORES)]
    res2 = run_bass_kernel_spmd(nc2, in_maps, core_ids=list(range(NCORES)))
    yfull = np.empty((B * S, DIM), np.float32)
    for c in range(NCORES):
        yfull[perm[c]] = res2.results[c]["y"]
    return yfull.reshape(B, S, DIM)
